# revision 5
# baseline (speedup 1.0000x reference)
"""Trainium2 Bass kernel for BasePropagationGraphPositionalEncoding.

Computes, for each batch element b:
    out[b] = (sum_k coefs[k] * gr_kernel[b, k]) @ x[b] / sum_k coefs[k]
with coefs[k] = (1 - EPS)^k, EPS = 0.01, K = 9.

Sharding: batch dim B=8 across the 8 NeuronCores (data parallel, no
cross-core communication).

v2: gr_kernel (and x) are cast to bf16 on the host before staging, halving
the HBM stream from 36 MB to 18 MB per core (the memory-bound term; output
tolerance is 2e-2, bf16 keeps rel err ~2e-3). The weighted k-sum is split
across engines so each fits under the per-band DMA window (~6.3 us):
  - DVE takes k=0..5: tensor_scalar scale (4x mode, bf16) + tensor_tensor
    add (2x mode). scalar_tensor_tensor is NOT used on DVE - it has no
    2x/4x uops and runs 1x (10.1 us/band, over budget).
  - Pool (gpsimd) takes k=6..8 with its own accumulator via STT.
  - PE merges the two partial accumulators for free: both are transposed
    into the same PSUM tile with matmul-accumulate (is_transpose + start/
    stop), then contracted against x with PSUM-accumulated matmuls.
"""

import sys

if "/opt/trn_rl_repo" not in sys.path:
    sys.path.insert(0, "/opt/trn_rl_repo")

import ml_dtypes
import numpy as np

import concourse.bass as bass
import concourse.mybir as mybir
from concourse import tile
from concourse.bacc import Bacc
from concourse.masks import make_identity
from concourse.bass_utils import run_bass_kernel_spmd

# Problem shapes (hardcoded per the harness contract).
B, K, N, D = 8, 9, 1024, 64
EPS = 0.01
P = 128          # SBUF partitions
NT = N // P      # 8 row/col tiles of the [N, N] kernel

F32 = mybir.dt.float32
BF16 = mybir.dt.bfloat16
NP_BF16 = ml_dtypes.bfloat16

# Slab -> engine split: DVE gets k=0..5, Pool k=6..8.
DVE_KS = list(range(6))
POOL_KS = [6, 7, 8]
# Slab pairs per DMA: 5 transfers per band of 512/256 KB each.
PAIRS = [(0, 2), (2, 2), (4, 2), (6, 2), (8, 1)]


def build_bass() -> bass.Bass:
    # Bacc (not plain Bass): its compile() runs generate_event_semaphores /
    # move_matmul_waits_to_ldweights, splitting multi-semaphore waits that
    # the 64B ISA instructions (single EVENTS slot) cannot carry.
    nc = Bacc()

    x_d = nc.dram_tensor("x_b", (N, D), BF16, kind="ExternalInput")
    g_d = nc.dram_tensor("g_b", (K, N, N), BF16, kind="ExternalInput")
    o_d = nc.dram_tensor("out_b", (N, D), F32, kind="ExternalOutput")

    coefs = (1.0 - EPS) ** np.arange(K, dtype=np.float64)
    w = coefs / coefs.sum()  # fold the 1/sum normalization into the k-sum

    with tile.TileContext(nc) as tc:
        with (
            tc.tile_pool(name="consts", bufs=1) as consts,
            tc.tile_pool(name="gr", bufs=3) as gr_pool,
            tc.tile_pool(name="accv", bufs=2) as accv_pool,
            tc.tile_pool(name="accp", bufs=2) as accp_pool,
            tc.tile_pool(name="scr", bufs=2) as scr_pool,
            tc.tile_pool(name="wkt", bufs=2) as wkt_pool,
            tc.tile_pool(name="outp", bufs=2) as out_pool,
            tc.tile_pool(name="ps_t", bufs=4, space=bass.MemorySpace.PSUM) as ps_t,
            tc.tile_pool(name="ps_e", bufs=2, space=bass.MemorySpace.PSUM) as ps_e,
        ):
            # Per-band paired-slab loads: pair p of band i is one contiguous-
            # per-partition DMA ([128, nk, 1024] bf16 <- [nk, 128, 1024]
            # DRAM view, 2 KB runs). 5 DMAs per band instead of 9 halves the
            # SP HWDGE issue cost (565 ns each).
            def load_band(i, engines=None):
                tiles = []
                for p, (k0, nk) in enumerate(PAIRS):
                    g_t = gr_pool.tile([P, nk, N], BF16, tag=f"gp{p}",
                                       name=f"g{i}_{p}")
                    src = g_d[k0 : k0 + nk, i * P : (i + 1) * P, :].rearrange(
                        "k p m -> p k m"
                    )
                    eng = engines[p] if engines else nc.sync
                    eng.dma_start(g_t[:], src)
                    tiles.append(g_t)
                return tiles

            # Band 0: no dependency waits, so issue from three engines in
            # parallel to fill the SDMA queues faster during startup.
            band_tiles = load_band(
                0, engines=[nc.sync, nc.scalar, nc.sync, nc.gpsimd, nc.gpsimd]
            )

            # Identity for PE transpose (bf16, matching the accumulators).
            # Built by GPSIMD, then copied through VectorE so the first PE
            # transpose waits on a single semaphore (DVE).
            ident_raw = consts.tile([P, P], BF16)
            make_identity(nc, ident_raw)
            ident = consts.tile([P, P], BF16)
            nc.vector.tensor_copy(ident[:], ident_raw[:])

            # x rearranged to [p, chunk, d] so chunk c is a [128, 64] tile
            # with the contraction index m = c*128 + p on partitions.
            x_sb = consts.tile([P, NT, D], BF16)
            nc.gpsimd.dma_start(x_sb[:], x_d.rearrange("(c p) d -> p c d", p=P))

            def slab(tiles, k):
                for p, (k0, nk) in enumerate(PAIRS):
                    if k0 <= k < k0 + nk:
                        return tiles[p][:, k - k0, :]
                raise AssertionError(k)

            for i in range(NT):
                g_ts = band_tiles
                if i + 1 < NT:
                    band_tiles = load_band(i + 1)

                # DVE partial accumulator: k=0..5. Per slab: tensor_scalar
                # scale (4x) into scratch + tensor_tensor add (2x). Both
                # modes require all operands bf16/SBUF/unit-stride.
                acc_v = accv_pool.tile([P, N], BF16, tag="accv")
                nc.vector.tensor_scalar_mul(acc_v[:], slab(g_ts, 0), float(w[0]))
                for k in DVE_KS[1:]:
                    s = scr_pool.tile([P, N], BF16, tag="scr", name=f"s{i}_{k}")
                    nc.vector.tensor_scalar_mul(s[:], slab(g_ts, k), float(w[k]))
                    nc.vector.tensor_add(acc_v[:], acc_v[:], s[:])

                # Pool partial accumulator: k=6..8. Pool's ucode only has
                # tensor_scalar / tensor_tensor (TensorScalarPtr is not a
                # valid Pool opcode), so: Pool scales k6 itself, ACT
                # pre-scales k7/k8 (activation copy with scale), Pool adds
                # s7 and DVE adds s8 at the end of its chain.
                acc_p = accp_pool.tile([P, N], BF16, tag="accp")
                nc.gpsimd.tensor_scalar_mul(acc_p[:], slab(g_ts, 6), float(w[6]))
                s7 = scr_pool.tile([P, N], BF16, tag="scr7", name=f"s7_{i}")
                nc.scalar.mul(s7[:], slab(g_ts, 7), float(w[7]))
                nc.gpsimd.tensor_add(acc_p[:], acc_p[:], s7[:])
                s8 = scr_pool.tile([P, N], BF16, tag="scr8", name=f"s8_{i}")
                nc.scalar.mul(s8[:], slab(g_ts, 8), float(w[8]))
                nc.vector.tensor_add(acc_v[:], acc_v[:], s8[:])

                # Merge the two partial accumulators on DVE (CoreSim accepts
                # bf16 PSUM transpose-accumulate, but HW does not - PSUM
                # matmul-accumulate is fp32-only silicon), then transpose
                # each [128,128] chunk on PE and stage to SBUF via ACT.
                nc.vector.tensor_add(acc_v[:], acc_v[:], acc_p[:])
                wkT_sb = wkt_pool.tile([P, NT, P], BF16)
                for c in range(NT):
                    # bf16: transpose-mode PSUM out dtype must match lhsT.
                    ps = ps_t.tile([P, P], BF16)
                    nc.tensor.transpose(
                        ps[:], acc_v[:, c * P : (c + 1) * P], ident[:]
                    )
                    nc.scalar.copy(wkT_sb[:, c, :], ps[:])

                # emb[i-band] = sum_c wk_tile(i,c) @ x_chunk(c), accumulated
                # in PSUM over the 8 contraction chunks.
                emb_ps = ps_e.tile([P, D], F32)
                for c in range(NT):
                    nc.tensor.matmul(
                        emb_ps[:],
                        wkT_sb[:, c, :],
                        x_sb[:, c, :],
                        start=(c == 0),
                        stop=(c == NT - 1),
                    )

                o_sb = out_pool.tile([P, D], F32)
                nc.scalar.copy(o_sb[:], emb_ps[:])
                nc.sync.dma_start(o_d[i * P : (i + 1) * P, :], o_sb[:])

    nc.compile()
    return nc


_NC = None


def _get_nc() -> bass.Bass:
    global _NC
    if _NC is None:
        _NC = build_bass()
    return _NC


def run(x: np.ndarray, gr_kernel: np.ndarray, **spmd_kwargs):
    """Run the SPMD kernel on cores 0-7; returns BassKernelResults."""
    nc = _get_nc()
    x_bf = np.ascontiguousarray(x).astype(NP_BF16)
    g_bf = np.ascontiguousarray(gr_kernel).astype(NP_BF16)
    in_maps = [
        {"x_b": x_bf[b], "g_b": g_bf[b]}
        for b in range(B)
    ]
    return run_bass_kernel_spmd(nc, in_maps, core_ids=list(range(B)), **spmd_kwargs)


def kernel(x: np.ndarray, gr_kernel: np.ndarray) -> np.ndarray:
    res = run(np.asarray(x), np.asarray(gr_kernel))
    out = np.stack([res.results[b]["out_b"] for b in range(B)], axis=0)
    return out.astype(np.float32, copy=False)


if __name__ == "__main__":
    rng = np.random.default_rng(0)
    x = rng.standard_normal((B, N, D), dtype=np.float32)
    g = rng.standard_normal((B, K, N, N), dtype=np.float32)
    out = kernel(x, g)
    coefs = (1.0 - EPS) ** np.arange(K)
    wk = np.einsum("k,bknm->bnm", coefs, g)
    ref = np.matmul(wk, x) / coefs.sum()
    err = np.linalg.norm(out - ref) / np.linalg.norm(ref)
    print("self-check rel err:", err)


# revision 9
# speedup vs baseline: 2.2353x; 2.2353x over previous
"""Trainium2 Bass kernel for BasePropagationGraphPositionalEncoding.

Computes, for each batch element b:
    out[b] = (sum_k coefs[k] * gr_kernel[b, k]) @ x[b] / sum_k coefs[k]
with coefs[k] = (1 - EPS)^k, EPS = 0.01, K = 9.

Sharding: batch dim B=8 across the 8 NeuronCores (data parallel, no
cross-core communication).

v4: gr_kernel/x are cast to bf16 on the host before staging, halving the
HBM stream from 36 MB to 18 MB per core (tolerance is 2e-2; bf16 lands at
~5e-3). The weighted k-sum is split so each engine fits under the per-band
DMA window (~6.5-7 us at ~330 GB/s):

  - DVE (k=0..4 self-scaled, k=5,6 pre-scaled by ACT): tensor_scalar runs
    in 4x mode (~410 ns) and tensor_tensor in 2x mode (~680 ns) for bf16;
    scalar_tensor_tensor has NO fast uops (1x) so it is avoided.
  - PE (k=7,8): normal matmuls lhsT=G_k chunk, rhs=w_k*I accumulate
    w_k*G_k^T directly into f32 PSUM (per-chunk accumulation groups opened
    as the slabs arrive), closed by the accumulator transpose expressed as
    a normal matmul by the unscaled identity (bf16 transpose-mode PSUM
    accumulation is broken on HW; f32 normal-matmul accumulation is not).
  - ACT: scale-copies of k5/k6, 8 PSUM->SBUF chunk copies (f32->bf16), out.
  - Pool does NO compute and no DMA: its Q7 ucode ops are 6-30x slower
    than the cost model and stall concurrent DVE ops (shared SBUF port).
"""

import sys

if "/opt/trn_rl_repo" not in sys.path:
    sys.path.insert(0, "/opt/trn_rl_repo")

import ml_dtypes
import numpy as np

import concourse.bass as bass
import concourse.mybir as mybir
from concourse import tile
from concourse.bacc import Bacc
from concourse.masks import make_identity
from concourse.bass_utils import run_bass_kernel_spmd

# Problem shapes (hardcoded per the harness contract).
B, K, N, D = 8, 9, 1024, 64
EPS = 0.01
P = 128          # SBUF partitions
NT = N // P      # 8 row/col tiles of the [N, N] kernel

F32 = mybir.dt.float32
BF16 = mybir.dt.bfloat16
NP_BF16 = ml_dtypes.bfloat16

# Slab pairs per DMA: 5 transfers per band of 512/256 KB each.
PAIRS = [(0, 2), (2, 2), (4, 2), (6, 2), (8, 1)]


def build_bass() -> bass.Bass:
    # Bacc (not plain Bass): its compile() runs generate_event_semaphores /
    # move_matmul_waits_to_ldweights, splitting multi-semaphore waits that
    # the 64B ISA instructions (single EVENTS slot) cannot carry.
    nc = Bacc()

    x_d = nc.dram_tensor("x_b", (N, D), BF16, kind="ExternalInput")
    g_d = nc.dram_tensor("g_b", (K, N, N), BF16, kind="ExternalInput")
    o_d = nc.dram_tensor("out_b", (N, D), F32, kind="ExternalOutput")

    coefs = (1.0 - EPS) ** np.arange(K, dtype=np.float64)
    w = coefs / coefs.sum()  # fold the 1/sum normalization into the k-sum

    with tile.TileContext(nc) as tc:
        with (
            tc.tile_pool(name="consts", bufs=1) as consts,
            tc.tile_pool(name="gr", bufs=3) as gr_pool,
            tc.tile_pool(name="accv", bufs=2) as accv_pool,
            tc.tile_pool(name="scr", bufs=2) as scr_pool,
            tc.tile_pool(name="wkt", bufs=2) as wkt_pool,
            tc.tile_pool(name="outp", bufs=2) as out_pool,
            tc.tile_pool(name="ps_t", bufs=2, space=bass.MemorySpace.PSUM) as ps_t,
            tc.tile_pool(name="ps_e", bufs=2, space=bass.MemorySpace.PSUM) as ps_e,
        ):
            # Per-band paired-slab loads: pair p of band i is one DMA
            # ([128, nk, 1024] bf16 <- [nk, 128, 1024] DRAM view, 2 KB
            # contiguous runs). 5 DMAs per band keeps SP HWDGE issue cost
            # (~600 ns each) well under the band window. All loads go
            # through HWDGE (sync/scalar) - SWDGE (gpsimd) descriptor
            # rings contend with DVE's 2-port modes.
            def load_band(i, engines=None):
                tiles = []
                for p, (k0, nk) in enumerate(PAIRS):
                    g_t = gr_pool.tile([P, nk, N], BF16, tag=f"gp{p}",
                                       name=f"g{i}_{p}")
                    src = g_d[k0 : k0 + nk, i * P : (i + 1) * P, :].rearrange(
                        "k p m -> p k m"
                    )
                    eng = engines[p] if engines else nc.sync
                    eng.dma_start(g_t[:], src)
                    tiles.append(g_t)
                return tiles

            # Band 0: no dependency waits; spread issue over both HWDGE
            # engines to fill the SDMA queues faster during startup.
            band_tiles = load_band(
                0, engines=[nc.sync, nc.scalar, nc.sync, nc.scalar, nc.sync]
            )

            # Identities for the PE-side k-sum/transpose: plain I for the
            # accumulator transpose, w_k*I for the PE-owned slabs. Built by
            # GPSIMD once at startup, then staged through DVE (single-sem
            # dependencies for PE; also scales the diagonals in 4x mode).
            ident_raw = consts.tile([P, P], BF16)
            make_identity(nc, ident_raw)
            ident = consts.tile([P, P], BF16)
            nc.vector.tensor_copy(ident[:], ident_raw[:])
            wid7 = consts.tile([P, P], BF16)
            nc.vector.tensor_scalar_mul(wid7[:], ident_raw[:], float(w[7]))
            wid8 = consts.tile([P, P], BF16)
            nc.vector.tensor_scalar_mul(wid8[:], ident_raw[:], float(w[8]))

            # x rearranged to [p, chunk, d] so chunk c is a [128, 64] tile
            # with the contraction index m = c*128 + p on partitions.
            x_sb = consts.tile([P, NT, D], BF16)
            nc.scalar.dma_start(x_sb[:], x_d.rearrange("(c p) d -> p c d", p=P))

            def slab(tiles, k):
                for p, (k0, nk) in enumerate(PAIRS):
                    if k0 <= k < k0 + nk:
                        return tiles[p][:, k - k0, :]
                raise AssertionError(k)

            for i in range(NT):
                g_ts = band_tiles
                if i + 1 < NT:
                    band_tiles = load_band(i + 1)

                # PE-owned slabs: PSUM tiles are bank-granular, so pack 4
                # chunks into one [128, 512] f32 bank tile (2 halves per
                # band), each covered by ONE accumulation group opened as
                # soon as pair {6,7}/{8} arrives. Normal matmul:
                # psum[chunk cols] += (G_k chunk)^T @ (w_k I) = w_k G_k^T.
                pss = []
                for h in range(2):
                    ps = ps_t.tile([P, 4 * P], F32, tag=f"ps{h}", name=f"ps{i}_{h}")
                    for j, (kk, wid) in enumerate([(7, wid7), (8, wid8)]):
                        sl = slab(g_ts, kk)
                        for c4 in range(4):
                            c = h * 4 + c4
                            nc.tensor.matmul(
                                ps[:, c4 * P : (c4 + 1) * P],
                                sl[:, c * P : (c + 1) * P],
                                wid[:],
                                start=(j == 0 and c4 == 0),
                                stop=False,
                            )
                    pss.append(ps)

                # DVE accumulator: k=0..4 self-scaled (tensor_scalar 4x into
                # scratch + tensor_tensor 2x add), k=5/6 pre-scaled by ACT.
                acc_v = accv_pool.tile([P, N], BF16, tag="accv")
                nc.vector.tensor_scalar_mul(acc_v[:], slab(g_ts, 0), float(w[0]))
                for k in (1, 2, 3, 4):
                    s = scr_pool.tile([P, N], BF16, tag="scr", name=f"s{i}_{k}")
                    nc.vector.tensor_scalar_mul(s[:], slab(g_ts, k), float(w[k]))
                    nc.vector.tensor_add(acc_v[:], acc_v[:], s[:])
                for k in (5, 6):
                    s = scr_pool.tile([P, N], BF16, tag=f"sa{k}", name=f"s{i}_{k}")
                    nc.scalar.mul(s[:], slab(g_ts, k), float(w[k]))
                    nc.vector.tensor_add(acc_v[:], acc_v[:], s[:])

                # Close each half's group with the accumulator transposes,
                # expressed as normal matmuls by the unscaled identity
                # (f32 PSUM accumulate; bf16 transpose-mode accumulate is
                # broken on HW). Then ONE wide ACT copy per half stages
                # all 4 chunks to SBUF (f32->bf16).
                wkT_sb = wkt_pool.tile([P, NT, P], BF16)
                for h in range(2):
                    ps = pss[h]
                    for c4 in range(4):
                        c = h * 4 + c4
                        nc.tensor.matmul(
                            ps[:, c4 * P : (c4 + 1) * P],
                            acc_v[:, c * P : (c + 1) * P],
                            ident[:],
                            start=False,
                            stop=(c4 == 3),
                        )
                    nc.scalar.copy(wkT_sb[:, h * 4 : (h + 1) * 4, :], ps[:])

                # emb[i-band] = sum_c wk_tile(i,c) @ x_chunk(c), accumulated
                # in PSUM over the 8 contraction chunks.
                emb_ps = ps_e.tile([P, D], F32)
                for c in range(NT):
                    nc.tensor.matmul(
                        emb_ps[:],
                        wkT_sb[:, c, :],
                        x_sb[:, c, :],
                        start=(c == 0),
                        stop=(c == NT - 1),
                    )

                o_sb = out_pool.tile([P, D], F32)
                nc.scalar.copy(o_sb[:], emb_ps[:])
                nc.sync.dma_start(o_d[i * P : (i + 1) * P, :], o_sb[:])

    nc.compile()
    return nc


_NC = None


def _get_nc() -> bass.Bass:
    global _NC
    if _NC is None:
        _NC = build_bass()
    return _NC


def run(x: np.ndarray, gr_kernel: np.ndarray, **spmd_kwargs):
    """Run the SPMD kernel on cores 0-7; returns BassKernelResults."""
    nc = _get_nc()
    x_bf = np.ascontiguousarray(x).astype(NP_BF16)
    g_bf = np.ascontiguousarray(gr_kernel).astype(NP_BF16)
    in_maps = [
        {"x_b": x_bf[b], "g_b": g_bf[b]}
        for b in range(B)
    ]
    return run_bass_kernel_spmd(nc, in_maps, core_ids=list(range(B)), **spmd_kwargs)


def kernel(x: np.ndarray, gr_kernel: np.ndarray) -> np.ndarray:
    res = run(np.asarray(x), np.asarray(gr_kernel))
    out = np.stack([res.results[b]["out_b"] for b in range(B)], axis=0)
    return out.astype(np.float32, copy=False)


if __name__ == "__main__":
    rng = np.random.default_rng(0)
    x = rng.standard_normal((B, N, D), dtype=np.float32)
    g = rng.standard_normal((B, K, N, N), dtype=np.float32)
    out = kernel(x, g)
    coefs = (1.0 - EPS) ** np.arange(K)
    wk = np.einsum("k,bknm->bnm", coefs, g)
    ref = np.matmul(wk, x) / coefs.sum()
    err = np.linalg.norm(out - ref) / np.linalg.norm(ref)
    print("self-check rel err:", err)


# revision 10
# speedup vs baseline: 2.6764x; 1.1973x over previous
"""Trainium2 Bass kernel for BasePropagationGraphPositionalEncoding.

Computes, for each batch element b:
    out[b] = (sum_k coefs[k] * gr_kernel[b, k]) @ x[b] / sum_k coefs[k]
with coefs[k] = (1 - EPS)^k, EPS = 0.01, K = 9.

Sharding: batch dim B=8 across the 8 NeuronCores (data parallel, no
cross-core communication).

v5: gr_kernel/x are cast to bf16 on the host before staging, halving the
HBM stream from 36 MB to 18 MB per core (tolerance is 2e-2; bf16 lands at
~5e-3). Per-band work split so every engine fits under the per-band DMA
window (~6.5 us):

  - DMA: 9 per-slab transfers per band (256 KB, one contiguous 2 KB run
    per partition), striped across BOTH HWDGE rings (sync=SP ring, even k;
    scalar=ACT ring, odd k) - a single ring measured only ~282 GB/s.
  - DVE (k=0..5): tensor_scalar scale in 4x mode (~410 ns) + tensor_tensor
    add in 2x mode (~680 ns); scalar_tensor_tensor is avoided (no fast
    uops, 1x). ~5.9 us/band.
  - PE (k=6..8): normal matmuls lhsT=G_k chunk, rhs=w_k*I accumulate
    w_k*G_k^T into f32 PSUM ([128,512] bank tiles, 4 chunks each, one
    accumulation group per half opened as slabs arrive), closed by the
    DVE-accumulator transpose (matmul by unscaled I). bf16 transpose-mode
    PSUM accumulation is broken on HW; f32 normal-matmul accumulate works.
  - ACT: 2 wide PSUM->SBUF copies (f32->bf16) + out copy + odd-k DMA issue.
  - Pool (gpsimd) does NO compute/DMA: its Q7 ucode ops run 6-30x slower
    than the cost model and stall concurrent DVE ops (shared SBUF port).

The last band is processed in column halves (separate accumulators) so the
post-stream tail is halved: chunks 0-3 close/copy/matmul while DVE still
sums columns 512:1024.
"""

import sys

if "/opt/trn_rl_repo" not in sys.path:
    sys.path.insert(0, "/opt/trn_rl_repo")

import ml_dtypes
import numpy as np

import concourse.bass as bass
import concourse.mybir as mybir
from concourse import tile
from concourse.bacc import Bacc
from concourse.masks import make_identity
from concourse.bass_utils import run_bass_kernel_spmd

# Problem shapes (hardcoded per the harness contract).
B, K, N, D = 8, 9, 1024, 64
EPS = 0.01
P = 128          # SBUF partitions
NT = N // P      # 8 row/col tiles of the [N, N] kernel
H = N // 2

F32 = mybir.dt.float32
BF16 = mybir.dt.bfloat16
NP_BF16 = ml_dtypes.bfloat16

DVE_KS = (0, 1, 2, 3, 4, 5)   # DVE-owned slabs
PE_KS = (6, 7, 8)             # PE-owned slabs (diag-matmul k-sum)


def build_bass() -> bass.Bass:
    # Bacc (not plain Bass): its compile() runs generate_event_semaphores /
    # move_matmul_waits_to_ldweights, splitting multi-semaphore waits that
    # the 64B ISA instructions (single EVENTS slot) cannot carry.
    nc = Bacc()

    x_d = nc.dram_tensor("x_b", (N, D), BF16, kind="ExternalInput")
    g_d = nc.dram_tensor("g_b", (K, N, N), BF16, kind="ExternalInput")
    o_d = nc.dram_tensor("out_b", (N, D), F32, kind="ExternalOutput")

    coefs = (1.0 - EPS) ** np.arange(K, dtype=np.float64)
    w = coefs / coefs.sum()  # fold the 1/sum normalization into the k-sum

    with tile.TileContext(nc) as tc:
        with (
            tc.tile_pool(name="consts", bufs=1) as consts,
            tc.tile_pool(name="gr", bufs=3) as gr_pool,
            tc.tile_pool(name="accv", bufs=2) as accv_pool,
            tc.tile_pool(name="scr", bufs=2) as scr_pool,
            tc.tile_pool(name="wkt", bufs=2) as wkt_pool,
            tc.tile_pool(name="outp", bufs=2) as out_pool,
            tc.tile_pool(name="ps_t", bufs=2, space=bass.MemorySpace.PSUM) as ps_t,
            tc.tile_pool(name="ps_e", bufs=2, space=bass.MemorySpace.PSUM) as ps_e,
        ):
            # Per-band slab loads: one DMA per slab (contiguous 2 KB per
            # partition), striped across the two HWDGE rings so both DMA
            # queues stream concurrently.
            def load_band(i):
                tiles = []
                for k in range(K):
                    g_k = gr_pool.tile([P, N], BF16, tag=f"g{k}",
                                       name=f"g{i}_{k}")
                    eng = nc.sync if k % 2 == 0 else nc.scalar
                    eng.dma_start(g_k[:], g_d[k, i * P : (i + 1) * P, :])
                    tiles.append(g_k)
                return tiles

            band_tiles = load_band(0)

            # Identities for the PE-side k-sum/transpose: plain I for the
            # accumulator transpose, w_k*I for the PE-owned slabs. Built by
            # GPSIMD once at startup, then staged through DVE (single-sem
            # dependencies for PE; the 4x tensor_scalar also applies w_k).
            ident_raw = consts.tile([P, P], BF16)
            make_identity(nc, ident_raw)
            ident = consts.tile([P, P], BF16)
            nc.vector.tensor_copy(ident[:], ident_raw[:])
            wids = {}
            for k in PE_KS:
                wid = consts.tile([P, P], BF16, name=f"wid{k}")
                nc.vector.tensor_scalar_mul(wid[:], ident_raw[:], float(w[k]))
                wids[k] = wid

            # x rearranged to [p, chunk, d] so chunk c is a [128, 64] tile
            # with the contraction index m = c*128 + p on partitions.
            x_sb = consts.tile([P, NT, D], BF16)
            nc.scalar.dma_start(x_sb[:], x_d.rearrange("(c p) d -> p c d", p=P))

            for i in range(NT):
                last = i == NT - 1
                g_ts = band_tiles
                if i + 1 < NT:
                    band_tiles = load_band(i + 1)

                # PE-owned slabs: PSUM tiles are bank-granular, so pack 4
                # chunks into one [128, 512] f32 bank tile (2 halves per
                # band), each covered by ONE accumulation group opened as
                # soon as slab k=6 arrives. Normal matmul:
                # psum[chunk cols] += (G_k chunk)^T @ (w_k I) = w_k G_k^T.
                pss = []
                for h in range(2):
                    ps = ps_t.tile([P, 4 * P], F32, tag=f"ps{h}", name=f"ps{i}_{h}")
                    for j, kk in enumerate(PE_KS):
                        sl = g_ts[kk]
                        for c4 in range(4):
                            c = h * 4 + c4
                            nc.tensor.matmul(
                                ps[:, c4 * P : (c4 + 1) * P],
                                sl[:, c * P : (c + 1) * P],
                                wids[kk][:],
                                start=(j == 0 and c4 == 0),
                                stop=False,
                            )
                    pss.append(ps)

                # DVE accumulator (k=0..5): tensor_scalar (4x) into scratch
                # + tensor_tensor (2x) add. For the last band, build the
                # two column halves in separate accumulators so chunks 0-3
                # can close/copy/contract while columns 512+ still sum.
                halves = [(0, N, "accv")] if not last else [
                    (0, H, "acch0"), (H, N, "acch1")
                ]
                accs = []
                for lo, hi, tag in halves:
                    wdt = hi - lo
                    acc = accv_pool.tile([P, wdt], BF16, tag=tag,
                                         name=f"{tag}_{i}")
                    nc.vector.tensor_scalar_mul(
                        acc[:], g_ts[0][:, lo:hi], float(w[0])
                    )
                    for k in DVE_KS[1:]:
                        s = scr_pool.tile([P, wdt], BF16, tag=f"scr{tag}",
                                          name=f"s{tag}_{i}_{k}")
                        nc.vector.tensor_scalar_mul(
                            s[:], g_ts[k][:, lo:hi], float(w[k])
                        )
                        nc.vector.tensor_add(acc[:], acc[:], s[:])
                    accs.append((lo, acc))

                def acc_chunk(c):
                    # chunk c's [P, P] slice of the band accumulator(s)
                    for lo, acc in reversed(accs):
                        if c * P >= lo:
                            return acc[:, c * P - lo : (c + 1) * P - lo]
                    raise AssertionError(c)

                # Close each half's group with the accumulator transposes
                # (normal matmuls by unscaled I, f32 PSUM accumulate), then
                # ONE wide ACT copy per half stages 4 chunks to SBUF.
                wkT_sb = wkt_pool.tile([P, NT, P], BF16)
                for h in range(2):
                    ps = pss[h]
                    for c4 in range(4):
                        c = h * 4 + c4
                        nc.tensor.matmul(
                            ps[:, c4 * P : (c4 + 1) * P],
                            acc_chunk(c),
                            ident[:],
                            start=False,
                            stop=(c4 == 3),
                        )
                    nc.scalar.copy(wkT_sb[:, h * 4 : (h + 1) * 4, :], ps[:])

                # emb[i-band] = sum_c wk_tile(i,c) @ x_chunk(c), accumulated
                # in PSUM over the 8 contraction chunks.
                emb_ps = ps_e.tile([P, D], F32)
                for c in range(NT):
                    nc.tensor.matmul(
                        emb_ps[:],
                        wkT_sb[:, c, :],
                        x_sb[:, c, :],
                        start=(c == 0),
                        stop=(c == NT - 1),
                    )

                o_sb = out_pool.tile([P, D], F32)
                nc.scalar.copy(o_sb[:], emb_ps[:])
                nc.sync.dma_start(o_d[i * P : (i + 1) * P, :], o_sb[:])

    nc.compile()
    return nc


_NC = None


def _get_nc() -> bass.Bass:
    global _NC
    if _NC is None:
        _NC = build_bass()
    return _NC


def run(x: np.ndarray, gr_kernel: np.ndarray, **spmd_kwargs):
    """Run the SPMD kernel on cores 0-7; returns BassKernelResults."""
    nc = _get_nc()
    x_bf = np.ascontiguousarray(x).astype(NP_BF16)
    g_bf = np.ascontiguousarray(gr_kernel).astype(NP_BF16)
    in_maps = [
        {"x_b": x_bf[b], "g_b": g_bf[b]}
        for b in range(B)
    ]
    return run_bass_kernel_spmd(nc, in_maps, core_ids=list(range(B)), **spmd_kwargs)


def kernel(x: np.ndarray, gr_kernel: np.ndarray) -> np.ndarray:
    res = run(np.asarray(x), np.asarray(gr_kernel))
    out = np.stack([res.results[b]["out_b"] for b in range(B)], axis=0)
    return out.astype(np.float32, copy=False)


if __name__ == "__main__":
    rng = np.random.default_rng(0)
    x = rng.standard_normal((B, N, D), dtype=np.float32)
    g = rng.standard_normal((B, K, N, N), dtype=np.float32)
    out = kernel(x, g)
    coefs = (1.0 - EPS) ** np.arange(K)
    wk = np.einsum("k,bknm->bnm", coefs, g)
    ref = np.matmul(wk, x) / coefs.sum()
    err = np.linalg.norm(out - ref) / np.linalg.norm(ref)
    print("self-check rel err:", err)
